# revision 25
# baseline (speedup 1.0000x reference)
"""Trainium2 Bass kernel for nn_HeatmapEncoder.

Math per (b, s, c) and per coordinate set (gaze, hand):
    g = exp(-((gx-cx)^2 + (gy-cy)^2) / (2 sigma^2))   on a 336x336 grid
    g = g / (sum(g) + eps)            (zeroed when cx+cy <= 0)
    unified = g_gaze + g_hand
    out = unified / (max(unified) + eps)

The Gaussian is separable, so each unified map is rank-2: one K=2 bf16
matmul per 512-wide PSUM chunk (rows: (y_gaze, x_gaze), (y_hand,
x_hand); plain bf16 rounding, rel err ~4e-3 vs the 2e-2 gate).
Sum-normalization is folded into the y factors.

The peak is computed ANALYTICALLY before any map exists: for two
equal-sigma isotropic Gaussians the continuous max lies on the segment
between the two centers, so sampling
    f(t) = A exp(-D^2 t^2 / 2s^2) + B exp(-D^2 (1-t)^2 / 2s^2)
at SEG points (A, B = per-set amplitudes, D^2 = |c1-c2|^2 from the
host) bounds the grid peak to ~0.3%.  Gaze rows sample t^2, hand rows
(1-t)^2; a [64,32] pairing matmul adds the two sets per map, and a
[32,32] identity matmul broadcasts peak+eps across partitions — no
cross-partition DMA.  Each map is then drained ONCE from PSUM with a
fused scale (1/(peak+eps)) split as contiguous halves across DVE and
ACT so the PSUM slot frees fast, then DMA'd out on the sync queue (the
only deep DMA ring; gpsimd's ring hits ~1us DRAIN stalls when
overused, and a single queue already engages all 16 DMA engines).

Each map's four matmuls pack (c,x) into exactly 2 PSUM banks
([0:336]=c0, [336:512]+[512:672]=c1, [672:1008]=c2), allowing three
map tiles in flight so the PE does not stall on drains.  Factors are
computed on the natural [64, W] rows and scattered into the 32-aligned
PE layout with direct SBUF->SBUF DMAs (no DRAM bounce): map j = 4*b+q
keeps its 2 factor rows at partitions 32q, 32q+1, free block b
(LDWEIGHTS requires quadrant-aligned partition starts).  Map rows are
interleaved y = 3*p + c so each map is a single contiguous DRAM range
for the output DMA.

Steady state is bound by the per-core sustained HBM write rate
(~310 GB/s for the 14.45 MB/core output, ~47 us); the ~20 us head is
engine program load + preamble barrier + const DMAs + factor chain.

Sharding: pure data parallel over batch B=8 across the 8 cores.
"""

import functools
from contextlib import ExitStack

import numpy as np
import ml_dtypes

try:
    import concourse.bass as bass
except ImportError:  # pragma: no cover
    import sys

    sys.path.insert(0, "/opt/trn_rl_repo")
    import concourse.bass as bass

import concourse.tile as tile
from concourse import bacc, mybir
from concourse.bass_utils import run_bass_kernel_spmd

H = W = 336
P = 112  # partitions per y-chunk; y = 3*p + c  (c in 0..2)
NCH = 3
S_DIM, C_DIM = 8, 4
NMAPS = S_DIM * C_DIM  # 32 maps per core
NR = 2 * NMAPS  # 64 factor rows (map-major, gaze/hand interleaved)
NB = 8  # free blocks in the aligned factor layout (map j = 4*b + q)
N_CORES = 8
SIGMA = 10.0 / 336.0
EXP_SCALE = -1.0 / (2.0 * SIGMA * SIGMA)
EPS = 1e-6
SEG = 512  # segment samples for the analytic peak

F32 = mybir.dt.float32
BF16 = mybir.dt.bfloat16
AF = mybir.ActivationFunctionType
ALU = mybir.AluOpType
AX = mybir.AxisListType


def _emit(nc, tc, ctx, negc_in, dsq_in, out_t, grid_const, ts_const,
          pair_const, eye_const):
    const = ctx.enter_context(tc.tile_pool(name="const", bufs=1))
    fact = ctx.enter_context(tc.tile_pool(name="fact", bufs=1))
    ffac = ctx.enter_context(tc.tile_pool(name="ffac", bufs=1))
    small = ctx.enter_context(tc.tile_pool(name="small", bufs=2))
    sstage = ctx.enter_context(tc.tile_pool(name="sstage", bufs=14))
    pmap = ctx.enter_context(tc.tile_pool(name="pmap", bufs=3, space="PSUM"))
    ppk = ctx.enter_context(tc.tile_pool(name="ppk", bufs=1, space="PSUM"))

    # ---- early ACT table preload via dummy exp on a memset tile ----
    dum = small.tile([1, 16], F32, tag="dum")
    nc.vector.memset(dum[:], 0.0)
    dum2 = small.tile([1, 16], F32, tag="dum2")
    nc.scalar.activation(dum2[:], dum[:], AF.Exp, bias=0.0, scale=1.0)
    ONES = const.tile([NMAPS, P], BF16)
    nc.gpsimd.memset(ONES[:], 1.0)

    # ---- constants / inputs ----
    # sync: NC2 (tiny, first) + G gate the factor chain; rest on gpsimd
    NC2 = const.tile([NR, 2], F32)
    nc.sync.dma_start(NC2[:], negc_in.ap())
    G = const.tile([NR, W], F32)
    nc.sync.dma_start(G[:], grid_const.ap())
    PAIR = const.tile([NR, NMAPS], BF16)
    nc.gpsimd.dma_start(PAIR[:], pair_const.ap())
    EYE = const.tile([NMAPS, NMAPS], BF16)
    nc.gpsimd.dma_start(EYE[:], eye_const.ap())
    TS = const.tile([NR, SEG], F32)
    nc.gpsimd.dma_start(TS[:], ts_const.ap())
    DSQ = const.tile([NR, 1], F32)
    nc.gpsimd.dma_start(DSQ[:], dsq_in.ap())

    # ---- 1-D gaussian factors, [64, 336] fp32; row sums via ACT accum ----
    # y side first: it gates the FY scatters -> first matmul.
    # Normalization split per side: y carries valid/Sy, x carries 1/Sx
    # (the reference's +eps on Sx*Sy is ~1e-8 relative - dropped).
    sx = small.tile([NR, 1], F32, tag="sx")
    sy = small.tile([NR, 1], F32, tag="sy")
    sqy = fact.tile([NR, W], F32)
    nc.scalar.activation(sqy[:], G[:], AF.Square, bias=NC2[:, 1:2], scale=1.0)
    fyv = fact.tile([NR, W], F32)
    nc.scalar.activation(fyv[:], sqy[:], AF.Exp, bias=0.0, scale=EXP_SCALE,
                         accum_out=sy[:])
    sqx = fact.tile([NR, W], F32)
    nc.scalar.activation(sqx[:], G[:], AF.Square, bias=NC2[:, 0:1], scale=1.0)
    fxv = fact.tile([NR, W], F32)
    nc.scalar.activation(fxv[:], sqx[:], AF.Exp, bias=0.0, scale=EXP_SCALE,
                         accum_out=sx[:])

    # off the critical path: peak-segment arg + validity mask
    us = fact.tile([NR, SEG], F32)
    nc.vector.tensor_scalar_mul(us[:], TS[:], DSQ[:, 0:1])
    vs = small.tile([NR, 1], F32, tag="vs")
    nc.vector.tensor_add(vs[:], NC2[:, 0:1], NC2[:, 1:2])
    vm = small.tile([NR, 1], F32, tag="vm")  # valid: (-cx)+(-cy) < 0
    nc.vector.tensor_scalar(vm[:], vs[:], 0.0, None, op0=ALU.is_lt)

    # y factors: bf16 with valid/Sy folded in (critical path)
    ry = small.tile([NR, 1], F32, tag="ry")
    nc.vector.reciprocal(ry[:], sy[:])
    rv = small.tile([NR, 1], F32, tag="rv")
    nc.vector.tensor_mul(rv[:], ry[:], vm[:])
    yh = fact.tile([NR, W], BF16)
    nc.vector.tensor_scalar_mul(yh[:], fyv[:], rv[:, 0:1])

    # x factors: bf16 with 1/Sx folded in
    rx = small.tile([NR, 1], F32, tag="rx")
    nc.vector.reciprocal(rx[:], sx[:])
    xh = fact.tile([NR, W], BF16)
    nc.vector.tensor_scalar_mul(xh[:], fxv[:], rx[:, 0:1])

    # ---- analytic peak chain (no map read-back, no DRAM hop) ----
    # f_row(t_i) = av_row * exp(EXP_SCALE * D^2 * {t^2 | (1-t)^2})
    am = small.tile([NR, 1], F32, tag="am")
    nc.vector.tensor_mul(am[:], rx[:], ry[:])
    av = small.tile([NR, 1], F32, tag="av")
    nc.vector.tensor_mul(av[:], am[:], vm[:])
    ee = fact.tile([NR, SEG], F32)
    nc.scalar.activation(ee[:], us[:], AF.Exp, bias=0.0, scale=EXP_SCALE)
    pp = fact.tile([NR, SEG], BF16)
    nc.vector.tensor_scalar_mul(pp[:], ee[:], av[:, 0:1])
    # pair matmul: psum[j, i] = f_gaze(2j) + f_hand(2j+1) = unified on segment
    pk_ps = ppk.tile([NMAPS, SEG], F32, tag="pkps")
    nc.tensor.matmul(pk_ps[:], PAIR[:], pp[:], start=True, stop=True,
                     tile_position=(0, 0))
    pk = small.tile([NMAPS, 1], F32, tag="pk")
    nc.vector.reduce_max(pk[:], pk_ps[:], axis=AX.X)
    pke = small.tile([NMAPS, 1], F32, tag="pke")
    nc.vector.tensor_scalar_add(pke[:], pk[:], EPS)
    # broadcast pke across partitions via identity matmul, then 1/x
    pkb = small.tile([NMAPS, P], BF16, tag="pkb")
    nc.vector.tensor_scalar_mul(pkb[:], ONES[:], pke[:, 0:1])
    rg_ps = ppk.tile([P, NMAPS], F32, tag="rgps")
    nc.tensor.matmul(rg_ps[:], pkb[:], EYE[:], start=True, stop=True,
                     tile_position=(0, 0))
    RG = const.tile([P, NMAPS], F32)
    nc.vector.reciprocal(RG[:], rg_ps[:])

    # ---- PE warm-up: a few dummy matmuls right after the peak chain
    # keep the tensor engine streaming until the first map matmul, so
    # map 0 runs at the ramped clock instead of the cold p-state ----
    wps = ppk.tile([NMAPS, SEG], F32, tag="pkps")
    for _ in range(9):
        nc.tensor.matmul(wps[:, 0:P], ONES[:, 0:NMAPS], ONES[:],
                         start=True, stop=True, tile_position=(0, 0))

    # ---- scatter factors into the 32-aligned 2-row layout, SBUF->SBUF ----
    # (LDWEIGHTS requires quadrant-aligned partition starts, so the rows
    # must move to partitions 32q+t; per-q tiles keep deps fine-grained)
    FYq = [ffac.tile([128, NB, W], BF16, name=f"FY{q}", tag=f"fy{q}")
           for q in range(4)]
    FXq = [ffac.tile([128, NB, W], BF16, name=f"FX{q}", tag=f"fx{q}")
           for q in range(4)]
    for q in range(4):
        for t in range(2):
            nc.sync.dma_start(FYq[q][32 * q + t:32 * q + t + 1, :, :],
                              yh[2 * q + t::8, :])
            nc.gpsimd.dma_start(FXq[q][32 * q + t:32 * q + t + 1, :, :],
                                xh[2 * q + t::8, :])

    # DRAM view matching stage layout: out[m, y, x], y = 3p+c, z = 336c+x
    dview = out_t.ap().rearrange("m (p c) x -> p m (c x)", p=P)

    # 4 matmuls pack (c,x) flat into exactly 2 PSUM banks (1008 of 1024):
    # [0:336]=c0, [336:512]=c1 x<176, [512:672]=c1 x>=176, [672:1008]=c2.
    # 2-bank tiles allow pmap bufs=3, so the PE never stalls on drains.
    CHUNKS = ((0, 336, 0, 0), (336, 512, 1, 0), (512, 672, 1, 176),
              (672, 1008, 2, 0))

    def map_matmuls(j, pt):
        q, b = j % 4, j // 4
        for o0, o1, cix, x0 in CHUNKS:
            lhsT = FYq[q][32 * q:32 * q + 2, b, cix::3]
            rhs = FXq[q][32 * q:32 * q + 2, b, x0:x0 + (o1 - o0)]
            nc.tensor.matmul(pt[:, o0:o1], lhsT, rhs,
                             start=True, stop=True, tile_position=(32 * q, 0))

    HW2 = NCH * W // 2  # 504: contiguous drain halves (psum is packed)
    for j in range(NMAPS):
        pt = pmap.tile([P, 2 * 512], F32, tag="pmap")
        map_matmuls(j, pt)
        st = sstage.tile([P, NCH * W], F32, tag="sst")
        # fused scale-drain split across both engines frees the slot
        nc.vector.tensor_scalar_mul(st[:, 0:HW2], pt[:, 0:HW2],
                                    RG[:, j:j + 1])
        nc.scalar.mul(st[:, HW2:NCH * W], pt[:, HW2:NCH * W],
                      RG[:, j:j + 1])
        nc.sync.dma_start(dview[:, j:j + 1, :], st[:])


@functools.lru_cache(maxsize=1)
def _build():
    nc = bacc.Bacc("TRN2", target_bir_lowering=False, debug=False)
    negc_in = nc.dram_tensor("negc", [NR, 2], F32, kind="ExternalInput")
    dsq_in = nc.dram_tensor("dsq", [NR, 1], F32, kind="ExternalInput")
    out_t = nc.dram_tensor("out", [NMAPS, H, W], F32, kind="ExternalOutput")

    grid = (np.arange(W, dtype=np.float64) / (W - 1)).astype(np.float32)
    grid_const = nc.inline_tensor(np.tile(grid, (NR, 1)), name="gridc")

    t = np.arange(SEG, dtype=np.float64) / (SEG - 1)
    ts = np.empty((NR, SEG), dtype=np.float32)
    ts[0::2] = (t ** 2).astype(np.float32)
    ts[1::2] = ((1.0 - t) ** 2).astype(np.float32)
    ts_const = nc.inline_tensor(ts, name="tsc")

    pair = np.zeros((NR, NMAPS), dtype=ml_dtypes.bfloat16)
    pair[np.arange(NR), np.arange(NR) // 2] = 1
    pair_const = nc.inline_tensor(pair, name="pairc")
    eye_const = nc.inline_tensor(np.eye(NMAPS, dtype=ml_dtypes.bfloat16),
                                 name="eyec")

    with tile.TileContext(nc) as tc, ExitStack() as ctx:
        _emit(nc, tc, ctx, negc_in, dsq_in, out_t, grid_const, ts_const,
              pair_const, eye_const)
    nc.compile()
    return nc


def _in_map_for(gaze, hand, b):
    cg = np.asarray(gaze[b], dtype=np.float32).reshape(NMAPS, 2)
    ch = np.asarray(hand[b], dtype=np.float32).reshape(NMAPS, 2)
    inter = np.stack([cg, ch], axis=1).reshape(NR, 2)  # row 2*j + t
    d2 = ((cg - ch) ** 2).sum(-1)  # |c_gaze - c_hand|^2 per map
    dsq = np.repeat(d2, 2)[:, None].astype(np.float32)
    return {"negc": np.ascontiguousarray(-inter),
            "dsq": np.ascontiguousarray(dsq)}


def kernel(gaze_coords, hand_coords, _trace=False, **trace_kwargs):
    gaze_coords = np.asarray(gaze_coords, dtype=np.float32)
    hand_coords = np.asarray(hand_coords, dtype=np.float32)
    B = gaze_coords.shape[0]
    assert B == N_CORES, f"expected batch {N_CORES}, got {B}"
    nc = _build()
    in_maps = [_in_map_for(gaze_coords, hand_coords, b) for b in range(B)]
    res = run_bass_kernel_spmd(nc, in_maps, list(range(N_CORES)),
                               trace=_trace, **trace_kwargs)
    out = np.stack(
        [res.results[i]["out"].reshape(S_DIM, C_DIM, H, W) for i in range(B)],
        axis=0,
    ).astype(np.float32)
    if _trace:
        return out, res
    return out
